# revision 14
# baseline (speedup 1.0000x reference)
"""VQ codebook argmax kernel for Trainium2 (8 NeuronCores, SPMD data-parallel).

Problem: x [2,96,48,48,48] fp32, prototypes [512,96] fp32.
Output: argmax_k cosine_sim(x[:, :, v], prototypes[k]) -> [2,48,48,48] int32.

Math notes:
  - argmax over k of (x_hat . p_hat_k) == argmax over k of (x . p_hat_k):
    per-voxel positive scaling (1/||x||) never changes the argmax, so x is
    NOT normalized (saves a full partition-dim reduction on device).
  - prototypes ARE normalized (host side, fp32, same formula as reference).
  - matmul precision: one fp16 main matmul per tile (fp16 streams at the
    bf16 rate of 1 col/cycle on the PE, verified on HW), plus on EVERY
    OTHER tile a second fp16 matmul u@W folding both first-order
    correction terms: u = f16(xl + s*xh), W = f16((1+s)(Ph + Pl/s)) with
    xh = f16(x), xl = x - xh (exact in fp32), Ph = f16(pn), Pl = pn - Ph,
    s = 2^-6. Corrected tiles are ~exact (error 2^-20.7); uncorrected
    tiles carry plain-f16 error (2^-15.6). Cost: 1.5 passes/tile instead
    of 2 (the DVE argmax fold is the co-bottleneck, so the correction on
    even tiles is nearly free). Measured on the actual input: 58 argmax
    flips vs the fp32 reference (rel err 1.1e-2, gate is 2e-2).
    (Probed and rejected: fp8 DoubleRow streams 2N fp8 cols at 1 col/cycle
    on real HW (no 0.5 cycles/row despite the cost model); fp32r runs at
    ~2 cycles/col AND drops the HAM clock to half speed for the whole
    kernel.)
  - argmax on device: single fused custom DVE op per 128-voxel tile.
    The 512 sims live in one PSUM bank [128, 512]; columns are permuted so
    column q holds proto 511-2q and column 256+q holds proto 510-2q.
    The op consumes two 256-wide streams (Src0 = cols 0:256 from PSUM,
    Src1 = cols 256:512 staged into SBUF) and folds: m = max(a,b);
    rec = (m == running_max(m)); wo = (m == b);
    pos = (2(j+1) - 1024) + wo; accum MAX of select(rec, pos, -FLT_MAX).
    The accumulated A encodes the winner: k* = -(A + 511), and the
    (j asc, wo) priority order makes ties resolve EXACTLY like np.argmax
    (first occurrence) - fuzz-verified 20000 cases.
  - staging: tiles are processed in groups of 4 sharing one 4-bank PSUM
    tile [128, 4, 512]; a single Scalar ACTIVATE stages all 4 tiles'
    second halves ([128, 4, 256] strided across banks) to SBUF, paying the
    ~352-element Scalar pipeline-fill cost once per 4 tiles (~287ns/tile
    instead of ~446ns/tile).
  - warmup: the PE clock is throttled (~1.2 GHz) until ~3.4us of sustained
    activity. Warmup matmuls on a memset tile start as soon as the engines
    come up (~6us, before any input DMA lands) so the throttle releases
    before the first real matmul.
"""

import numpy as np
import ml_dtypes
from contextlib import ExitStack

import concourse.bass as bass
import concourse.bacc as bacc
import concourse.tile as tile
from concourse import mybir
from concourse.bass_utils import run_bass_kernel_spmd

# ----------------------------------------------------------------------------
# problem constants (hardcoded per contract)
N_CORES = 8
B, C, D, H, W = 2, 96, 48, 48, 48
N_VOX = B * D * H * W            # 221184
VOX_PER_CORE = N_VOX // N_CORES  # 27648
K = 512                          # prototypes
TILE_V = 128                     # voxels per matmul tile (PSUM partition dim)
TILES_PER_CORE = VOX_PER_CORE // TILE_V  # 216
CHUNK_V = 1024                   # voxels per DMA chunk
GROUP = 4                        # tiles per PSUM group / scalar staging copy

# ----------------------------------------------------------------------------
# custom DVE op registration (argmax fold over paired streams)

_VQARG_NAME = "VQ_ARGMAX_ANT"
_VQARG_OP = None


def _vqarg_reference(in0, in1, c0, c1, c2):
    a = np.asarray(in0, np.float32)
    b = np.asarray(in1, np.float32)
    p = a.shape[0]
    a2 = a.reshape(p, -1)
    b2 = b.reshape(p, -1)
    c1v = float(c1) if np.isscalar(c1) or isinstance(c1, float) else np.asarray(c1, np.float32)
    m = np.maximum(a2, b2)
    r = np.maximum.accumulate(m, axis=1)
    rec = m == r
    wo = (m == b2).astype(np.float32)
    n = a2.shape[1]
    s2 = (np.float32(-float(c2)) + np.float32(c1v) * np.arange(1, n + 1, dtype=np.float32))
    pos = s2[None, :] + wo
    body = np.where(rec, pos, np.float32(-3.4028235e38)).astype(np.float32)
    acc = body.max(axis=1, keepdims=True)
    return body.reshape(a.shape), acc


def _register_vqarg():
    global _VQARG_OP
    if _VQARG_OP is not None:
        return _VQARG_OP
    from concourse.dve_spec import (
        Spec, Src0, Src1, C1, C2, Zero, MaxNeg, eq, select, scan, AluOp, maxx,
        lower, _has_src1 as has_src1,
    )
    from concourse import dve_ops
    from concourse.dve_uop import DveOpSpec

    m = maxx(Src0, Src1)
    r = scan(AluOp.MAX, m)
    rec = eq(m, r)
    wo = eq(m, Src1)
    s2 = scan(AluOp.ADD, C1, init=Zero - C2)
    pos = s2 + wo
    spec = Spec(
        body=select(rec, pos, MaxNeg),
        accum=AluOp.MAX,
        reference=_vqarg_reference,
    )

    if _VQARG_NAME in dve_ops._SUB_OPCODE_FOR_NAME:
        row = dve_ops._SUB_OPCODE_FOR_NAME[_VQARG_NAME]
    else:
        row = max(dve_ops._SUB_OPCODE_FOR_NAME.values()) + 1
        assert row < 0x20, "no free custom-DVE opcode row"
        dve_ops._SUB_OPCODE_FOR_NAME[_VQARG_NAME] = row

    shas = {}
    for ver in ("v3", "v4"):
        s = DveOpSpec(
            name=_VQARG_NAME,
            opcode=row,
            uops=lower(spec, ver=ver),
            rd1_en=has_src1(spec),
        )
        shas[ver] = s.sha(ver)

    op = dve_ops.DveOp(_VQARG_NAME, spec, subdim=False, uops_sha=shas)
    if all(o.name != _VQARG_NAME for o in dve_ops.OPS):
        dve_ops.OPS.append(op)
    dve_ops.CUSTOM_DVE_SPECS[_VQARG_NAME] = spec
    _VQARG_OP = op
    return op


# ----------------------------------------------------------------------------
# device program

_PROG = None

import os as _os
N_WARMUP = int(_os.environ.get("VQ_WARMUP", "8"))
CORR_PERIOD = int(_os.environ.get("VQ_CORR_PERIOD", "2"))  # correct tiles with tid%P==0
S_COMB = 2.0 ** -6               # scale folding the two correction terms


def build_program(vox_per_core=VOX_PER_CORE, chunk_v=CHUNK_V):
    """Build + compile the per-core SPMD Bass program. Returns nc."""
    vqarg = _register_vqarg()
    dt = mybir.dt
    n_tiles = vox_per_core // TILE_V
    n_corr = (n_tiles + CORR_PERIOD - 1) // CORR_PERIOD

    nc = bacc.Bacc(
        "TRN2", target_bir_lowering=False, debug=False, num_devices=N_CORES
    )
    xh_d = nc.dram_tensor("xh", [C, vox_per_core], dt.float16, kind="ExternalInput").ap()
    uc_d = nc.dram_tensor("uc", [C, n_corr * TILE_V], dt.float16, kind="ExternalInput").ap()
    ph_d = nc.dram_tensor("pht", [C, K], dt.float16, kind="ExternalInput").ap()
    wt_d = nc.dram_tensor("wt", [C, K], dt.float16, kind="ExternalInput").ap()
    out_d = nc.dram_tensor("outA", [TILE_V, n_tiles], dt.float32, kind="ExternalOutput").ap()

    with tile.TileContext(nc) as tc, ExitStack() as ctx:
        cpool = ctx.enter_context(tc.tile_pool(name="const", bufs=1))
        xpool = ctx.enter_context(tc.tile_pool(name="x", bufs=3))
        ppool = ctx.enter_context(tc.tile_pool(name="psum", bufs=2, space="PSUM"))
        spool = ctx.enter_context(tc.tile_pool(name="scr", bufs=3))
        hpool = ctx.enter_context(tc.tile_pool(name="half", bufs=3))
        apool = ctx.enter_context(tc.tile_pool(name="acc", bufs=1))

        # PE warmup on memset data: no input dependency, so it starts as
        # soon as the engines come up and releases the HAM clock throttle
        # (~3.4us sustained PE activity -> 2.4 GHz) before real matmul 0.
        if N_WARMUP:
            wsrc = cpool.tile([TILE_V, K], dt.bfloat16)
            nc.gpsimd.memset(wsrc[:], 0.0)
            wps = ppool.tile([TILE_V, GROUP, K], dt.float32, tag="ps")
            for _ in range(N_WARMUP):
                nc.tensor.matmul(wps[:, 0:1, :], wsrc[:, 0:TILE_V], wsrc[:],
                                 start=True, stop=True)

        # prototype tables on the gpsimd DMA queue so they land in parallel
        # with the first x chunk on the sync queue
        ph_sb = cpool.tile([C, K], dt.float16)
        nc.gpsimd.dma_start(ph_sb[:], ph_d[:])
        wt_sb = cpool.tile([C, K], dt.float16)
        nc.gpsimd.dma_start(wt_sb[:], wt_d[:])

        jsb = apool.tile([TILE_V, n_tiles], dt.float32)

        # ramp-in: smaller leading chunks so the first matmul starts sooner
        chunks = (vox_per_core - 1024) // chunk_v
        sizes = [512, 512] + [chunk_v] * chunks
        assert sum(sizes) == vox_per_core
        base = 0
        tid = 0
        pend = []
        for cv in sizes:
            xh_sb = xpool.tile([C, cv], dt.float16, tag="xh")
            nc.sync.dma_start(xh_sb[:], xh_d[:, base:base + cv])
            # corrected tiles within this chunk (global tid % CORR_PERIOD == 0)
            t0 = base // TILE_V
            ctiles = [t for t in range(t0, t0 + cv // TILE_V) if t % CORR_PERIOD == 0]
            if ctiles:
                uc_sb = xpool.tile([C, len(ctiles) * TILE_V], dt.float16, tag="uc")
                u0 = (ctiles[0] // CORR_PERIOD) * TILE_V
                nc.sync.dma_start(
                    uc_sb[:], uc_d[:, u0:u0 + len(ctiles) * TILE_V])
            base += cv
            for t in range(cv // TILE_V):
                pend.append((tid, xh_sb[:, t * TILE_V:(t + 1) * TILE_V],
                             None if tid % CORR_PERIOD else
                             uc_sb[:, ctiles.index(t0 + t) * TILE_V:
                                   (ctiles.index(t0 + t) + 1) * TILE_V]))
                tid += 1
                if len(pend) < GROUP:
                    continue
                ps4 = ppool.tile([TILE_V, GROUP, K], dt.float32, tag="ps")
                for j, (gt, lhs, lhsu) in enumerate(pend):
                    if lhsu is None:
                        nc.tensor.matmul(ps4[:, j:j + 1, :], lhs, ph_sb[:],
                                         start=True, stop=True)
                    else:
                        nc.tensor.matmul(ps4[:, j:j + 1, :], lhs, ph_sb[:],
                                         start=True, stop=False)
                        nc.tensor.matmul(ps4[:, j:j + 1, :], lhsu, wt_sb[:],
                                         start=False, stop=True)
                # one Scalar ACTIVATE stages all 4 tiles' second halves
                # (flat tile: 2D in1 slices keep the TTSS struct / imm2 slot)
                half = hpool.tile([TILE_V, GROUP * (K // 2)], dt.float32)
                nc.scalar.copy(half[:], ps4[:, :, K // 2:K])
                for j, (gt, _, _) in enumerate(pend):
                    scr = spool.tile([TILE_V, K // 2], dt.float32)
                    nc.vector._custom_dve(
                        vqarg,
                        out=scr[:],
                        in0=ps4[:, j:j + 1, 0:K // 2],
                        in1=half[:, j * (K // 2):(j + 1) * (K // 2)],
                        s0=0.0,
                        s1=2.0,
                        imm2=1024.0,
                        accum_out=jsb[:, gt:gt + 1],
                    )
                pend = []
                if n_tiles > 32 and tid == n_tiles - 24:
                    # drain most results early (hidden under remaining tiles)
                    # so only a 12KB DMA sits after the last fold
                    nc.sync.dma_start(out_d[:, :tid], jsb[:, :tid])
        assert tid == n_tiles and not pend
        split = n_tiles - 24 if n_tiles > 32 else 0
        nc.sync.dma_start(out_d[:, split:], jsb[:, split:])

    nc.compile()
    return nc


def _get_program():
    global _PROG
    if _PROG is None:
        _PROG = build_program()
    return _PROG


# ----------------------------------------------------------------------------
# host-side prep + entry point

def _prep_x(xt_core):
    """f16 hi part [C, V] + packed correction u for tiles with tid%P==0."""
    hi = xt_core.astype(np.float16)
    u = ((xt_core - hi.astype(np.float32))
         + S_COMB * hi.astype(np.float32)).astype(np.float16)
    ut = u.reshape(C, -1, TILE_V)[:, ::CORR_PERIOD, :].reshape(C, -1)
    return hi, np.ascontiguousarray(ut)


def _prep_prototypes(prototypes):
    pn = prototypes / np.maximum(
        np.linalg.norm(prototypes, axis=1, keepdims=True), 1e-12
    )
    pn = pn.astype(np.float32)
    q = np.arange(K // 2)
    perm = np.concatenate([511 - 2 * q, 510 - 2 * q])  # col layout for VQARG
    pc = pn[perm]
    ph = pc.astype(np.float16)
    pl = pc - ph.astype(np.float32)
    w = ((1.0 + S_COMB) * (ph.astype(np.float32) + pl / S_COMB)).astype(np.float16)
    return np.ascontiguousarray(ph.T), np.ascontiguousarray(w.T)  # [96, 512] f16


def kernel(x, prototypes):
    x = np.asarray(x, np.float32)
    prototypes = np.asarray(prototypes, np.float32)

    # [2,96,48,48,48] -> [96, 221184] with global voxel = b*110592 + dhw
    xt = np.ascontiguousarray(
        x.reshape(B, C, D * H * W).transpose(1, 0, 2).reshape(C, N_VOX)
    )
    pht, wt = _prep_prototypes(prototypes)

    in_maps = []
    for c in range(N_CORES):
        sl = slice(c * VOX_PER_CORE, (c + 1) * VOX_PER_CORE)
        xh_c, uc_c = _prep_x(np.ascontiguousarray(xt[:, sl]))
        in_maps.append({
            "xh": xh_c,
            "uc": uc_c,
            "pht": pht,
            "wt": wt,
        })

    nc = _get_program()
    res = None
    last_err = None
    for attempt in range(3):
        try:
            res = run_bass_kernel_spmd(nc, in_maps, list(range(N_CORES)))
            break
        except Exception as e:  # transient axon/NRT hiccups self-recover
            last_err = e
            import time as _time
            _time.sleep(20 * (attempt + 1))
    if res is None:
        raise last_err

    outs = []
    for c in range(N_CORES):
        A = np.asarray(res.results[c]["outA"], np.float32)  # [128, 216]
        kidx = -(A + np.float32(511.0))                     # exact small ints
        outs.append(kidx.T.reshape(-1))                     # voxel = t*128 + p
    full = np.concatenate(outs)
    return full.reshape(B, D, H, W).astype(np.int32)


# revision 17
# speedup vs baseline: 1.4804x; 1.4804x over previous
"""VQ codebook argmax kernel for Trainium2 (8 NeuronCores, SPMD data-parallel).

Problem: x [2,96,48,48,48] fp32, prototypes [512,96] fp32.
Output: argmax_k cosine_sim(x[:, :, v], prototypes[k]) -> [2,48,48,48] int32.

Math notes:
  - argmax over k of (x_hat . p_hat_k) == argmax over k of (x . p_hat_k):
    per-voxel positive scaling (1/||x||) never changes the argmax, so x is
    NOT normalized (saves a full partition-dim reduction on device).
  - prototypes ARE normalized (host side, fp32, same formula as reference).
  - matmul precision: TWO fp16 matmuls (fp16 runs at bf16 rate on the PE,
    verified on HW): sims' = xh@Ph + u@W with xh=f16(x), xl=x-xh (exact in
    fp32), u=f16(xl + s*xh), Ph=f16(pn), Pl=pn-Ph, W=f16((1+s)(Ph+Pl/s)),
    s=2^-6. Algebra: sims' = (1+s+s^2)*sims + O(2^-20.7); the global
    (1+s+s^2) scale never changes the argmax. Measured on the actual
    input: 1 argmax flip vs fp32 reference (rel err 4e-4, gate is 2e-2).
  - argmax on device: single fused custom DVE op per 128-voxel tile.
    The 512 sims live in one PSUM bank [128, 512]; columns are permuted so
    column q holds proto 511-2q and column 256+q holds proto 510-2q.
    The op consumes two 256-wide streams (Src0 = cols 0:256 from PSUM,
    Src1 = cols 256:512 via an SBUF copy done by the Scalar engine) and
    folds: m = max(a,b); rec = (m == running_max(m)); wo = (m == b);
    pos = (2(j+1) - 1024) + wo; accum MAX of select(rec, pos, -FLT_MAX).
    The accumulated A encodes the winner: k* = -(A + 511), and the
    (j asc, wo) priority order makes ties resolve EXACTLY like np.argmax
    (first occurrence) - fuzz-verified 20000 cases.
"""

import numpy as np
import ml_dtypes
from contextlib import ExitStack

import concourse.bass as bass
import concourse.bacc as bacc
import concourse.tile as tile
from concourse import mybir
from concourse.bass_utils import run_bass_kernel_spmd

# ----------------------------------------------------------------------------
# problem constants (hardcoded per contract)
N_CORES = 8
B, C, D, H, W = 2, 96, 48, 48, 48
N_VOX = B * D * H * W            # 221184
VOX_PER_CORE = N_VOX // N_CORES  # 27648
K = 512                          # prototypes
TILE_V = 128                     # voxels per matmul tile (PSUM partition dim)
TILES_PER_CORE = VOX_PER_CORE // TILE_V  # 216
CHUNK_V = 1024                   # voxels per DMA chunk
CHUNKS = VOX_PER_CORE // CHUNK_V  # 27
TILES_PER_CHUNK = CHUNK_V // TILE_V  # 8

_BF16 = ml_dtypes.bfloat16
S_COMB = 2.0 ** -6               # scale folding the two correction terms

# ----------------------------------------------------------------------------
# custom DVE op registration (argmax fold over paired streams)

_VQARG_NAME = "VQ_ARGMAX_ANT"
_VQARG_OP = None


def _vqarg_reference(in0, in1, c0, c1, c2):
    a = np.asarray(in0, np.float32)
    b = np.asarray(in1, np.float32)
    p = a.shape[0]
    a2 = a.reshape(p, -1)
    b2 = b.reshape(p, -1)
    c1v = float(c1) if np.isscalar(c1) or isinstance(c1, float) else np.asarray(c1, np.float32)
    m = np.maximum(a2, b2)
    r = np.maximum.accumulate(m, axis=1)
    rec = m == r
    wo = (m == b2).astype(np.float32)
    n = a2.shape[1]
    s2 = (np.float32(-float(c2)) + np.float32(c1v) * np.arange(1, n + 1, dtype=np.float32))
    pos = s2[None, :] + wo
    body = np.where(rec, pos, np.float32(-3.4028235e38)).astype(np.float32)
    acc = body.max(axis=1, keepdims=True)
    return body.reshape(a.shape), acc


def _register_vqarg():
    global _VQARG_OP
    if _VQARG_OP is not None:
        return _VQARG_OP
    from concourse.dve_spec import (
        Spec, Src0, Src1, C1, C2, Zero, MaxNeg, eq, select, scan, AluOp, maxx,
        lower, _has_src1 as has_src1,
    )
    from concourse import dve_ops
    from concourse.dve_uop import DveOpSpec

    m = maxx(Src0, Src1)
    r = scan(AluOp.MAX, m)
    rec = eq(m, r)
    wo = eq(m, Src1)
    s2 = scan(AluOp.ADD, C1, init=Zero - C2)
    pos = s2 + wo
    spec = Spec(
        body=select(rec, pos, MaxNeg),
        accum=AluOp.MAX,
        reference=_vqarg_reference,
    )

    if _VQARG_NAME in dve_ops._SUB_OPCODE_FOR_NAME:
        row = dve_ops._SUB_OPCODE_FOR_NAME[_VQARG_NAME]
    else:
        row = max(dve_ops._SUB_OPCODE_FOR_NAME.values()) + 1
        assert row < 0x20, "no free custom-DVE opcode row"
        dve_ops._SUB_OPCODE_FOR_NAME[_VQARG_NAME] = row

    shas = {}
    for ver in ("v3", "v4"):
        s = DveOpSpec(
            name=_VQARG_NAME,
            opcode=row,
            uops=lower(spec, ver=ver),
            rd1_en=has_src1(spec),
        )
        shas[ver] = s.sha(ver)

    op = dve_ops.DveOp(_VQARG_NAME, spec, subdim=False, uops_sha=shas)
    if all(o.name != _VQARG_NAME for o in dve_ops.OPS):
        dve_ops.OPS.append(op)
    dve_ops.CUSTOM_DVE_SPECS[_VQARG_NAME] = spec
    _VQARG_OP = op
    return op


# ----------------------------------------------------------------------------
# device program

_PROG = None

import os as _os
ACT_COPY = _os.environ.get("VQ_ACT_COPY", "1") == "1"
N_WARMUP = int(_os.environ.get("VQ_WARMUP", "8"))
GROUP = 2                        # tiles per PSUM group / scalar staging copy


def build_program(vox_per_core=VOX_PER_CORE, chunk_v=CHUNK_V):
    """Build + compile the per-core SPMD Bass program. Returns (nc, meta)."""
    vqarg = _register_vqarg()
    dt = mybir.dt
    chunks = vox_per_core // chunk_v
    tiles_per_chunk = chunk_v // TILE_V
    n_tiles = vox_per_core // TILE_V

    nc = bacc.Bacc(
        "TRN2", target_bir_lowering=False, debug=False, num_devices=N_CORES
    )
    xh_d = nc.dram_tensor("xh", [C, vox_per_core], dt.float16, kind="ExternalInput").ap()
    xl_d = nc.dram_tensor("xl", [C, vox_per_core], dt.float16, kind="ExternalInput").ap()
    ph_d = nc.dram_tensor("pht", [C, K], dt.float16, kind="ExternalInput").ap()
    pl_d = nc.dram_tensor("plt", [C, K], dt.float16, kind="ExternalInput").ap()
    out_d = nc.dram_tensor("outA", [TILE_V, n_tiles], dt.float32, kind="ExternalOutput").ap()

    with tile.TileContext(nc) as tc, ExitStack() as ctx:
        cpool = ctx.enter_context(tc.tile_pool(name="const", bufs=1))
        xpool = ctx.enter_context(tc.tile_pool(name="x", bufs=3))
        ppool = ctx.enter_context(tc.tile_pool(name="psum", bufs=4, space="PSUM"))
        spool = ctx.enter_context(tc.tile_pool(name="scr", bufs=3))
        hpool = ctx.enter_context(tc.tile_pool(name="half", bufs=3))
        apool = ctx.enter_context(tc.tile_pool(name="acc", bufs=1))

        # PE warmup on memset data: no input dependency, so it starts as
        # soon as the engines come up (~6us) and releases the HAM clock
        # throttle (~3.4us sustained PE activity -> 2.4 GHz) right as the
        # first real matmul's inputs land. 8 x 512-col f16 matmuls at the
        # throttled 1.2 GHz = 3.4us exactly. Results are discarded.
        if N_WARMUP:
            wsrc = cpool.tile([TILE_V, K], dt.float16)
            nc.gpsimd.memset(wsrc[:], 0.0)
            wps = ppool.tile([TILE_V, GROUP, K], dt.float32, tag="ps")
            for _ in range(N_WARMUP):
                nc.tensor.matmul(wps[:, 0:1, :], wsrc[:, 0:TILE_V], wsrc[:],
                                 start=True, stop=True)

        # tables go on the gpsimd DMA queue so they land in parallel with the
        # first x chunk on the sync queue (PE needs both before matmul 0)
        ph_sb = cpool.tile([C, K], dt.float16)
        nc.gpsimd.dma_start(ph_sb[:], ph_d[:])
        pl_sb = cpool.tile([C, K], dt.float16)
        nc.gpsimd.dma_start(pl_sb[:], pl_d[:])

        jsb = apool.tile([TILE_V, n_tiles], dt.float32)

        # ramp-in: small leading chunks so the first matmul starts sooner
        if chunks > 2:
            sizes = [256, 256, 512] + [chunk_v] * (chunks - 1)
        else:
            sizes = [chunk_v] * chunks
        assert sum(sizes) == vox_per_core
        base = 0
        tid = 0
        pend = []
        for cv in sizes:
            xh_sb = xpool.tile([C, cv], dt.float16, tag="xh")
            nc.sync.dma_start(xh_sb[:], xh_d[:, base:base + cv])
            xl_sb = xpool.tile([C, cv], dt.float16, tag="xl")
            nc.sync.dma_start(xl_sb[:], xl_d[:, base:base + cv])
            base += cv
            for t in range(cv // TILE_V):
                pend.append((tid, xh_sb[:, t * TILE_V:(t + 1) * TILE_V],
                             xl_sb[:, t * TILE_V:(t + 1) * TILE_V]))
                tid += 1
                if len(pend) < GROUP:
                    continue
                psg = ppool.tile([TILE_V, GROUP, K], dt.float32, tag="ps")
                for j, (gt, lhs_h, lhs_l) in enumerate(pend):
                    nc.tensor.matmul(psg[:, j:j + 1, :], lhs_h, ph_sb[:],
                                     start=True, stop=False)
                    nc.tensor.matmul(psg[:, j:j + 1, :], lhs_l, pl_sb[:],
                                     start=False, stop=True)
                # one Scalar ACTIVATE stages the group's second halves
                # ([128, GROUP, 256] strided across banks), paying the
                # ~352-elem Scalar pipeline-fill cost once per group.
                # (flat tile: 2D in1 slices keep the TTSS struct / imm2 slot)
                half = hpool.tile([TILE_V, GROUP * (K // 2)], dt.float32)
                nc.scalar.copy(half[:], psg[:, :, K // 2:K])
                for j, (gt, _, _) in enumerate(pend):
                    scr = spool.tile([TILE_V, K // 2], dt.float32)
                    nc.vector._custom_dve(
                        vqarg,
                        out=scr[:],
                        in0=psg[:, j:j + 1, 0:K // 2],
                        in1=half[:, j * (K // 2):(j + 1) * (K // 2)],
                        s0=0.0,
                        s1=2.0,
                        imm2=1024.0,
                        accum_out=jsb[:, gt:gt + 1],
                    )
                pend = []
                if n_tiles > 32 and tid == n_tiles - 8:
                    # drain most results early (hidden under remaining tiles)
                    # so only a 4KB DMA sits after the last fold
                    nc.sync.dma_start(out_d[:, :tid], jsb[:, :tid])
        assert tid == n_tiles and not pend
        split = n_tiles - 8 if n_tiles > 32 else 0
        nc.sync.dma_start(out_d[:, split:], jsb[:, split:])

    nc.compile()
    return nc


def _get_program():
    global _PROG
    if _PROG is None:
        _PROG = build_program()
    return _PROG


# ----------------------------------------------------------------------------
# host-side prep + entry point

def _bf16_split(a):
    """fp16 hi part + fp16 combined-correction part u = f16(xl + s*xh)."""
    hi = a.astype(np.float16)
    lo = ((a - hi.astype(np.float32)) + S_COMB * hi.astype(np.float32)).astype(
        np.float16
    )
    return hi, lo


def _prep_prototypes(prototypes):
    pn = prototypes / np.maximum(
        np.linalg.norm(prototypes, axis=1, keepdims=True), 1e-12
    )
    pn = pn.astype(np.float32)
    q = np.arange(K // 2)
    perm = np.concatenate([511 - 2 * q, 510 - 2 * q])  # col layout for VQARG
    pc = pn[perm]
    ph = pc.astype(np.float16)
    pl = pc - ph.astype(np.float32)
    w = ((1.0 + S_COMB) * (ph.astype(np.float32) + pl / S_COMB)).astype(np.float16)
    pht = np.ascontiguousarray(ph.T)  # [96, 512] f16
    plt = np.ascontiguousarray(w.T)   # [96, 512] f16 (the combined W table)
    return pht, plt


def kernel(x, prototypes):
    x = np.asarray(x, np.float32)
    prototypes = np.asarray(prototypes, np.float32)

    # [2,96,48,48,48] -> [96, 221184] with global voxel = b*110592 + dhw
    xt = np.ascontiguousarray(
        x.reshape(B, C, D * H * W).transpose(1, 0, 2).reshape(C, N_VOX)
    )
    xh, xl = _bf16_split(xt)
    pht, plt = _prep_prototypes(prototypes)

    in_maps = []
    for c in range(N_CORES):
        sl = slice(c * VOX_PER_CORE, (c + 1) * VOX_PER_CORE)
        in_maps.append({
            "xh": np.ascontiguousarray(xh[:, sl]),
            "xl": np.ascontiguousarray(xl[:, sl]),
            "pht": pht,
            "plt": plt,
        })

    nc = _get_program()
    res = None
    last_err = None
    for attempt in range(3):
        try:
            res = run_bass_kernel_spmd(nc, in_maps, list(range(N_CORES)))
            break
        except Exception as e:  # transient axon/NRT hiccups self-recover
            last_err = e
            import time as _time
            _time.sleep(20 * (attempt + 1))
    if res is None:
        raise last_err

    outs = []
    for c in range(N_CORES):
        A = np.asarray(res.results[c]["outA"], np.float32)  # [128, 216]
        kidx = -(A + np.float32(511.0))                     # exact small ints
        outs.append(kidx.T.reshape(-1))                     # voxel = t*128 + p
    full = np.concatenate(outs)
    return full.reshape(B, D, H, W).astype(np.int32)

